# revision 34
# baseline (speedup 1.0000x reference)
"""TRN2 Bass kernel for nn_MAD_4612794876395 (retrieval_knn).

Math: with dist = softmax_k(-||pos_d - pos_r||) and sum_k dist = 1, the
reference output collapses to
    out[b,c] = wmem@adapt_w + adapt_b + wdiff@field_b.reshape(H,C)
             + sum_h wdiff[b,h] * (date@field_w)[b, h*C+c]
where wdiff[b,h] = sum_k dist[b,k]*diff[b,k,h].  The dominant term is the
137 GFLOP date@field_w product, computed on 8 NeuronCores tensor-parallel
over field_w's 65536 columns (64 h-values per core) as fp16 matmuls at
1 row/cycle.

The h-contraction (one multiply-add per matmul output element) is the
hard part: per-partition-scalar ops force 128-wide tiles and the three
elementwise engines together cannot sustain 4 such ops per 853ns matmul
chain.  Instead field_w columns are reordered c-major/h-minor on the
host so each PSUM tile is [128b, 8c x 64h], and a custom DVE op
(out = running sum of in0*in1, one elem/cycle) computes weighted prefix
sums in a single 512-wide pass; the 8 segment ends are DMA-extracted
and the host finishes with a cheap difference.  Small terms are host
numpy.

v2 scheduling: the warm matmul stream already runs at the 216 ns/MM
hardware floor, so the wins are all at the edges:
 - startup DMA triggers cost ~650 ns each on the serialized Sync queue,
   which is busy with framework preamble until ~7.3 us.  The Activation
   queue (also hardware-DGE) is free at ~6.0 us and otherwise unused, so
   the startup-critical loads go there as a few large partition-major
   packed transfers ordered by consumption priority.
 - fw slice prefetch is one 512 KB trigger per slice (4 KB packets)
   on the Sync queue instead of 4x128 KB.
 - the PE clock is HAM-throttled to 1.2 GHz until ~3.4 us of sustained
   activity: a run of dummy matmuls on a zeroed tile spans the DMA wait
   so the real matmuls start at full 2.4 GHz.
"""
import sys

sys.path.insert(0, "/opt/trn_rl_repo")

import numpy as np

N_DATA, F, H, C, K, B = 100000, 512, 512, 128, 8, 2048
NCORES = 8
HSH = H // NCORES          # 64 h-values per core
SH = HSH * C               # 8192 field_w cols per core
P = 128
NB = B // P                # 16 b-tiles
NS = SH // 512             # 16 col-slices of 512 (8 c-segments x 64 h)
N_DUMMY = 22               # HAM warmup matmuls (N=256, ~213 ns each cold)
NHYB = 1                   # hybrid slices: f<256 fp16 + f>=256 fp8
NFULL = 3                  # full-fp8 slices: all 512 f in fp8 (2 DR MMs)
NH = NHYB + NFULL
NF = NS - NH               # pure-fp16 slices
# fp8 slices interleaved among fp16 ones: their chains outpace the DVE
# prefix-sum (~540/663 ns vs ~690 ns), so the DVE lag they build must
# drain during fp16 groups (853 ns cadence) before the next fp8 group
FULL_SLICES = (4, 8, 12)
HYB_SLICES = (14,)
F16_SLICES = tuple(n for n in range(NS)
                   if n not in FULL_SLICES and n not in HYB_SLICES)
SC_D, SC_W = 32.0, 512.0   # fp8 operand scales; product 2**14
SC = SC_D * SC_W

_NC = None
_LAST_IN_MAPS = None

_WSUM_SHAS = {"v3": "b3fc3e78a862b7eb", "v4": "bc6a002865d48b97"}


def _register_wsum():
    """Register the weighted-prefix-sum custom DVE op (idempotent)."""
    from concourse import dve_ops
    from concourse.dve_spec import Spec, Src0, Src1, scan, AluOp

    name = "ANT_WSUM_SCAN"
    for op in dve_ops.OPS:
        if op.name == name:
            return op

    def ref(in0, in1, s0, s1, imm2):
        p0 = in0.astype(np.float32).reshape(in0.shape[0], -1)
        p1 = in1.astype(np.float32).reshape(in1.shape[0], -1)
        return np.cumsum(p0 * p1, axis=-1).reshape(in0.shape)

    spec = Spec(body=scan(AluOp.ADD, Src0 * Src1), reference=ref)
    op = dve_ops.DveOp(name, spec, subdim=False, uops_sha=dict(_WSUM_SHAS))
    dve_ops.OPS.append(op)
    dve_ops._SUB_OPCODE_FOR_NAME[name] = (
        max(dve_ops._SUB_OPCODE_FOR_NAME.values()) + 1)
    assert dve_ops._SUB_OPCODE_FOR_NAME[name] < 0x20
    return op


def _build():
    import concourse.bass as bass
    import concourse.mybir as mybir
    import concourse.tile as tile
    from concourse import bacc

    wsum = _register_wsum()

    nc = bacc.Bacc(None, target_bir_lowering=False, debug=False)
    # dateT packed [p, t, fc, j] = date[t*128+j, fc*128+p]: every t-range
    # DMA piece is contiguous in HBM with 1 KB runs, and the stationary
    # slice dall[:, t, fc, :] is a contiguous [128, 128] AP
    dh = nc.dram_tensor("dh", [P, NB, 4, P], mybir.dt.float16,
                        kind="ExternalInput")
    # fw slice 0 partition-major: fw0[p, fc*512+c] = fw2[fc*128+p, c]
    fw0 = nc.dram_tensor("fw0", [P, 4 * 512], mybir.dt.float16,
                         kind="ExternalInput")
    # fw slices 1..NF-1: fwn[(n-1)*128+p, fc*512+c] = fw2[fc*128+p, n*512+c]
    fwn = nc.dram_tensor("fwn", [(NF - 1) * P, 4 * 512], mybir.dt.float16,
                         kind="ExternalInput")
    # hybrid slices: f<256 half in fp16 pre-scaled by 2**14, f>=256 half
    # quantized to fp8e4 (scales 32*512 = 2**14); full-fp8 slices have all
    # four f-chunks in fp8.  The DVE weights for both carry the inverse
    # scale.
    fwh16 = nc.dram_tensor("fwh16", [NHYB * P, 2 * 512], mybir.dt.float16,
                           kind="ExternalInput")
    fwh8 = nc.dram_tensor("fwh8", [NHYB * P, 2 * 512], mybir.dt.float8e4,
                          kind="ExternalInput")
    fwf8 = nc.dram_tensor("fwf8", [NFULL * P, 4 * 512], mybir.dt.float8e4,
                          kind="ExternalInput")
    # date fp8 pairs: dh8[p, t, fc, m] = e4m3(date[t*128+m, fc*128+p]*32)
    dh8 = nc.dram_tensor("dh8", [P, NB, 4, P], mybir.dt.float8e4,
                         kind="ExternalInput")
    wdsh = nc.dram_tensor("wdsh", [P, NB * HSH], mybir.dt.float32,
                          kind="ExternalInput")
    # wdiff b-tiled: [p, t, h] = wdiff[t*128+p, h] -> contiguous col-splits
    wds = nc.dram_tensor("wds", [P, NB * HSH], mybir.dt.float32,
                         kind="ExternalInput")
    # prefix-sum segment ends; host differences them into per-c sums
    ends = nc.dram_tensor("ends", [B, C], mybir.dt.float32,
                          kind="ExternalOutput")

    with tile.TileContext(nc) as tc:
        with (
            tc.tile_pool(name="const", bufs=1) as cp,
            tc.tile_pool(name="fwp", bufs=6) as fwp,
            tc.tile_pool(name="wgp", bufs=8) as wgp,
            tc.tile_pool(name="ps", bufs=7, space="PSUM") as ps,
            tc.tile_pool(name="dps", bufs=1, space="PSUM") as dps,
        ):
            # resident fp16 dateT, wdiff rows, SBUF staging for segment ends
            dall = cp.tile([P, NB, 4, P], mybir.dt.float16, name="dall")
            f0 = cp.tile([P, 4 * 512], mybir.dt.float16, name="f0")
            # wdiff in two tiles so chain-0's DVE never depends on the
            # later wdiff DMA piece
            wra = cp.tile([P, 4, HSH], mybir.dt.float32, name="wra")
            wrc = cp.tile([P, NB - 4, HSH], mybir.dt.float32, name="wrc")
            es = [cp.tile([P, C], mybir.dt.float32, name=f"es{t}")
                  for t in range(NB)]
            dum = cp.tile([P, 256], mybir.dt.float16, name="dum")
            dum_ps = dps.tile([P, 256], mybir.dt.float32, name="dum_ps")
            # fp8-slice operands (resident; needed only from ~halfway)
            dall8 = cp.tile([P, NB, 4, P], mybir.dt.float8e4, name="dall8")
            fh16 = [cp.tile([P, 2 * 512], mybir.dt.float16, name=f"fh16_{j}")
                    for j in range(NHYB)]
            fh8 = [cp.tile([P, 2, 512], mybir.dt.float8e4, name=f"fh8_{j}")
                   for j in range(NHYB)]
            ff8 = [cp.tile([P, 4, 512], mybir.dt.float8e4, name=f"ff8_{j}")
                   for j in range(NFULL)]
            wrh = cp.tile([P, NB, HSH], mybir.dt.float32, name="wrh")

            # per-b-tile wdiff rows repeated 8x via a stride-0 middle dim
            wrb, wrbh = [], []
            for t in range(NB):
                s = (wra[:, t:t + 1, :] if t < 4
                     else wrc[:, t - 4:t - 3, :])
                wrb.append(bass.AP(s.tensor, s.offset,
                                   [s.ap[0], [0, 8], s.ap[-1]]))
                sh = wrh[:, t:t + 1, :]
                wrbh.append(bass.AP(sh.tensor, sh.offset,
                                    [sh.ap[0], [0, 8], sh.ap[-1]]))

            # HAM warmup: dummy matmuls on a zeroed tile span the startup
            # DMA wait so real matmuls begin at the full 2.4 GHz clock
            nc.gpsimd.memset(dum[:], 0.0)
            for _ in range(N_DUMMY):
                nc.tensor.matmul(dum_ps[:], dum[:, 0:128], dum[:],
                                 start=True, stop=True)

            # input loads interleaved across the two HWDGE queues (Sync +
            # Activation) so descriptor generation runs in parallel; pieces
            # split so each chain's dependencies land just in time
            nc.sync.dma_start(dall[:, 0, :, :], dh[:, 0, :, :])
            nc.scalar.dma_start(f0[:], fw0[:])
            nc.sync.dma_start(dall[:, 1, :, :], dh[:, 1, :, :])
            nc.scalar.dma_start(dall[:, 2:4, :, :], dh[:, 2:4, :, :])
            nc.sync.dma_start(wra[:], wds[:, 0:4 * HSH])
            nc.scalar.dma_start(dall[:, 4:8, :, :], dh[:, 4:8, :, :])
            nc.sync.dma_start(dall[:, 8:12, :, :], dh[:, 8:12, :, :])
            nc.scalar.dma_start(dall[:, 12:NB, :, :], dh[:, 12:NB, :, :])
            nc.sync.dma_start(wrc[:], wds[:, 4 * HSH:])

            # fw slice prefetch, one 512 KB trigger per slice, alternating
            # queues
            fts = [f0]
            for n in range(1, NF):
                ft = fwp.tile([P, 4 * 512], mybir.dt.float16, name="ft",
                              tag="ft")
                eng = nc.scalar if n % 2 else nc.sync
                eng.dma_start(ft[:], fwn[(n - 1) * P:n * P, :])
                fts.append(ft)
            # fp8-slice loads (const tiles, consumed from ~halfway on)
            nc.sync.dma_start(dall8[:], dh8[:])
            nc.scalar.dma_start(wrh[:], wdsh[:])
            for j in range(NHYB):
                eng = nc.scalar if j % 2 else nc.sync
                eng.dma_start(fh16[j][:], fwh16[j * P:(j + 1) * P, :])
                eng2 = nc.sync if j % 2 else nc.scalar
                eng2.dma_start(fh8[j][:], fwh8[j * P:(j + 1) * P, :])
            for j in range(NFULL):
                eng = nc.scalar if j % 2 else nc.sync
                eng.dma_start(ff8[j][:], fwf8[j * P:(j + 1) * P, :])

            def emit_chain(n, t):
                hyb = n in HYB_SLICES
                full = n in FULL_SLICES
                g = ps.tile([P, 512], mybir.dt.float32, name="g", tag="g")
                if hyb:
                    j = HYB_SLICES.index(n)
                    for fc in range(2):
                        nc.tensor.matmul(
                            g[:], dall[:, t, fc, :],
                            fh16[j][:, fc * 512:(fc + 1) * 512],
                            start=(fc == 0), stop=False)
                    nc.tensor.matmul(
                        g[:], dall8[:, t, 2:4, :], fh8[j][:],
                        start=False, stop=True,
                        perf_mode=mybir.MatmulPerfMode.DoubleRow)
                elif full:
                    j = FULL_SLICES.index(n)
                    for half in range(2):
                        nc.tensor.matmul(
                            g[:], dall8[:, t, 2 * half:2 * half + 2, :],
                            ff8[j][:, 2 * half:2 * half + 2, :],
                            start=(half == 0), stop=(half == 1),
                            perf_mode=mybir.MatmulPerfMode.DoubleRow)
                else:
                    j = F16_SLICES.index(n)
                    for fc in range(4):
                        nc.tensor.matmul(
                            g[:], dall[:, t, fc, :],
                            fts[j][:, fc * 512:(fc + 1) * 512],
                            start=(fc == 0), stop=(fc == 3))
                # weighted prefix sum over the tile in one DVE pass
                wg = wgp.tile([P, 8, HSH], mybir.dt.float32, name="wg",
                              tag="wg")
                nc.vector._custom_dve(wsum, out=wg[:], in0=g[:],
                                      in1=(wrbh if (hyb or full)
                                           else wrb)[t])
                # stage segment ends in SBUF (GPSIMD is otherwise idle)
                nc.gpsimd.tensor_copy(es[t][:, n * 8:(n + 1) * 8],
                                      wg[:, :, HSH - 1:HSH])
                if n == NS - 1:
                    nc.scalar.dma_start(ends[t * P:(t + 1) * P, :],
                                        es[t][:])

            # full-fp8 slices are chain-interleaved with the preceding fp16
            # slice: the fp16 chain's 864 ns of streaming hides the two
            # 256-column DoubleRow LDWEIGHTS, and the DVE sees a smoothed
            # (853+540)/2 cadence instead of a sustained 540 ns burst
            sched = [0, 1, 2, (3, 4), 5, 6, (7, 8), 9, 10, (11, 12),
                     13, 14, 15]
            for item in sched:
                if isinstance(item, tuple):
                    a, b = item
                    for t in range(NB):
                        emit_chain(a, t)
                        emit_chain(b, t)
                else:
                    for t in range(NB):
                        emit_chain(item, t)
    nc.finalize()
    return nc


def kernel(idx, date, train_dates, mem, train_nns, pos_w, pos_b, field_w,
           field_b, adapt_w, adapt_b):
    global _NC, _LAST_IN_MAPS
    from concourse.bass_utils import run_bass_kernel_spmd

    idx = np.asarray(idx)
    date = np.asarray(date, dtype=np.float32)
    train_dates = np.asarray(train_dates, dtype=np.float32)
    mem = np.asarray(mem, dtype=np.float32)
    train_nns = np.asarray(train_nns)
    pos_w = np.asarray(pos_w, dtype=np.float32)
    pos_b = np.asarray(pos_b, dtype=np.float32)
    field_w = np.asarray(field_w, dtype=np.float32)
    field_b = np.asarray(field_b, dtype=np.float32)
    adapt_w = np.asarray(adapt_w, dtype=np.float32)
    adapt_b = np.asarray(adapt_b, dtype=np.float32)

    # ---- host phase 1 (small): dist, wdiff, const terms ----
    refs = train_nns[idx]                                   # [B, K]
    pos_d = date @ pos_w + pos_b                            # [B, H]
    pos_r = (train_dates[refs.reshape(-1)] @ pos_w + pos_b).reshape(B, K, H)
    diff = pos_d[:, None, :] - pos_r                        # [B, K, H]
    norm = np.sqrt((diff * diff).sum(-1))                   # [B, K]
    m = norm.min(axis=1, keepdims=True)
    e = np.exp(m - norm)
    dist = e / e.sum(axis=1, keepdims=True)                 # [B, K]
    wdiff = np.einsum("bk,bkh->bh", dist, diff).astype(np.float32)
    wmem = np.einsum("bk,bkc->bc", dist, mem[refs]).astype(np.float32)
    const = wmem @ adapt_w + adapt_b + wdiff @ field_b.reshape(H, C)

    # ---- device phase 2: grad-term, TP over the 65536 dim ----
    if _NC is None:
        _NC = _build()
    import ml_dtypes
    f8dt = ml_dtypes.float8_e4m3

    def q8(x, scale):
        return np.clip(x * scale, -235.0, 235.0).astype(f8dt)

    dateT16 = date.T.astype(np.float16)                     # [F, B]
    date4 = dateT16.reshape(4, P, NB, P)                    # [fc, p, t, j]
    dh = np.ascontiguousarray(date4.transpose(1, 2, 0, 3))  # [p, t, fc, j]
    dateTf = date.T.reshape(4, P, NB, P)                    # fp32 [fc, p, t, j]
    dh8 = np.ascontiguousarray(
        q8(dateTf.transpose(1, 2, 0, 3), SC_D))             # [p, t, fc, m]
    fw3 = field_w.reshape(F, H, C)                          # [f, h, c]
    in_maps = []
    for i in range(NCORES):
        # c-major/h-minor columns for this core's h range
        fw2f = np.ascontiguousarray(
            fw3[:, i * HSH:(i + 1) * HSH, :].transpose(0, 2, 1)
        ).reshape(F, SH)                                    # col = c*64 + h
        fw4f = fw2f.reshape(4, P, NS, 512)                  # [fc, p, n, c]
        fw4 = fw4f.astype(np.float16)
        fw0 = np.ascontiguousarray(
            fw4[:, :, 0, :].transpose(1, 0, 2)).reshape(P, 4 * 512)
        fwn = np.ascontiguousarray(
            fw4[:, :, list(F16_SLICES[1:]), :].transpose(2, 1, 0, 3)
        ).reshape((NF - 1) * P, 4 * 512)
        fwh16 = np.ascontiguousarray(
            (fw4f[0:2, :, list(HYB_SLICES), :] * SC).transpose(2, 1, 0, 3)
        ).astype(np.float16).reshape(NHYB * P, 2 * 512)     # [n, p, fc, c]
        fwh8 = np.ascontiguousarray(
            q8(fw4f[2:4, :, list(HYB_SLICES), :], SC_W).transpose(2, 1, 0, 3)
        ).reshape(NHYB * P, 2 * 512)                        # [n, p, i, c]
        fwf8 = np.ascontiguousarray(
            q8(fw4f[:, :, list(FULL_SLICES), :], SC_W).transpose(2, 1, 0, 3)
        ).reshape(NFULL * P, 4 * 512)                       # [n, p, fc, c]
        # wdiff b-tiled [p, t, h] = wdiff[t*128+p, i*64+h]
        wdt = np.ascontiguousarray(
            wdiff[:, i * HSH:(i + 1) * HSH]
            .reshape(NB, P, HSH).transpose(1, 0, 2).reshape(P, NB * HSH))
        in_maps.append({
            "dh": dh,
            "dh8": dh8,
            "fw0": fw0,
            "fwn": fwn,
            "fwh16": fwh16,
            "fwh8": fwh8,
            "fwf8": fwf8,
            "wds": wdt,
            "wdsh": (wdt / np.float32(SC)).astype(np.float32),
        })
    _LAST_IN_MAPS = in_maps
    res = run_bass_kernel_spmd(_NC, in_maps, core_ids=list(range(NCORES)))
    grad_term = np.zeros((B, C), dtype=np.float32)
    for i in range(NCORES):
        e8 = res.results[i]["ends"].reshape(B, NS, 8)
        grad_term += np.diff(e8, axis=2, prepend=0.0).reshape(B, C)
    return (const + grad_term).astype(np.float32)


def run_device(trace=False):
    """Re-run the device phase on the last inputs (test.py profiling)."""
    from concourse.bass_utils import run_bass_kernel_spmd
    assert _NC is not None and _LAST_IN_MAPS is not None
    return run_bass_kernel_spmd(_NC, _LAST_IN_MAPS,
                                core_ids=list(range(NCORES)), trace=trace)


# revision 35
# speedup vs baseline: 1.0015x; 1.0015x over previous
"""TRN2 Bass kernel for nn_MAD_4612794876395 (retrieval_knn).

Math: with dist = softmax_k(-||pos_d - pos_r||) and sum_k dist = 1, the
reference output collapses to
    out[b,c] = wmem@adapt_w + adapt_b + wdiff@field_b.reshape(H,C)
             + sum_h wdiff[b,h] * (date@field_w)[b, h*C+c]
where wdiff[b,h] = sum_k dist[b,k]*diff[b,k,h].  The dominant term is the
137 GFLOP date@field_w product, computed on 8 NeuronCores tensor-parallel
over field_w's 65536 columns (64 h-values per core) as fp16 matmuls at
1 row/cycle.

The h-contraction (one multiply-add per matmul output element) is the
hard part: per-partition-scalar ops force 128-wide tiles and the three
elementwise engines together cannot sustain 4 such ops per 853ns matmul
chain.  Instead field_w columns are reordered c-major/h-minor on the
host so each PSUM tile is [128b, 8c x 64h], and a custom DVE op
(out = running sum of in0*in1, one elem/cycle) computes weighted prefix
sums in a single 512-wide pass; the 8 segment ends are DMA-extracted
and the host finishes with a cheap difference.  Small terms are host
numpy.

Optimizations over the first working version (242.5 us -> ~216 us):
 - Scheduling: the warm fp16 matmul stream runs at the 216 ns/MM
   hardware floor, so the wins are at the edges.  Startup loads are
   split into consumption-ordered pieces across both HWDGE queues
   (Sync + Activation) for parallel descriptor generation; every piece
   is a single large partition-major packed transfer (1-4 KB packets).
   A run of dummy matmuls on a zeroed tile spans the startup DMA wait
   so the HAM clock gate (PE at 1.2 GHz until ~3.4 us of sustained
   activity) is already open when the real matmuls start.
 - Mixed precision: the correctness budget (rel err < 2e-2) is spent on
   fp8 DoubleRow matmuls (2 fp8 MACs/cell/cycle).  3 of 16 column
   slices run all 512 f-channels in fp8e4 (two K=256-pair DR matmuls,
   ~540 ns/chain vs 853), 1 slice runs half fp16 / half fp8 (~663 ns).
   e4m3 operand scales 32 (date) x 512 (field_w) = 2**14 are folded
   into the pure-fp16 half (pre-scaled x2**14) and into a second
   wdiff/2**14 DVE weight set, so the PSUM accumulation stays
   consistent.  Measured rel err 1.77e-2 (numpy fp8 sim matches
   hardware to 4 digits); pure fp16 would be 2.9e-4.
 - The fp8 chains outpace the ~690 ns DVE prefix-sum, so fp8 slices are
   interleaved chain-by-chain with fp16 slices: the fp16 streaming
   hides the 256-column DR LDWEIGHTS and the DVE lag drains during
   fp16 groups instead of piling into the kernel tail.
"""
import sys

sys.path.insert(0, "/opt/trn_rl_repo")

import numpy as np

N_DATA, F, H, C, K, B = 100000, 512, 512, 128, 8, 2048
NCORES = 8
HSH = H // NCORES          # 64 h-values per core
SH = HSH * C               # 8192 field_w cols per core
P = 128
NB = B // P                # 16 b-tiles
NS = SH // 512             # 16 col-slices of 512 (8 c-segments x 64 h)
N_DUMMY = 22               # HAM warmup matmuls (N=256, ~213 ns each cold)
NHYB = 1                   # hybrid slices: f<256 fp16 + f>=256 fp8
NFULL = 3                  # full-fp8 slices: all 512 f in fp8 (2 DR MMs)
NH = NHYB + NFULL
NF = NS - NH               # pure-fp16 slices
# fp8 slices interleaved among fp16 ones: their chains outpace the DVE
# prefix-sum (~540/663 ns vs ~690 ns), so the DVE lag they build must
# drain during fp16 groups (853 ns cadence) before the next fp8 group
FULL_SLICES = (4, 8, 12)
HYB_SLICES = (14,)
F16_SLICES = tuple(n for n in range(NS)
                   if n not in FULL_SLICES and n not in HYB_SLICES)
SC_D, SC_W = 32.0, 512.0   # fp8 operand scales; product 2**14
SC = SC_D * SC_W

_NC = None
_LAST_IN_MAPS = None

_WSUM_SHAS = {"v3": "b3fc3e78a862b7eb", "v4": "bc6a002865d48b97"}


def _register_wsum():
    """Register the weighted-prefix-sum custom DVE op (idempotent)."""
    from concourse import dve_ops
    from concourse.dve_spec import Spec, Src0, Src1, scan, AluOp

    name = "ANT_WSUM_SCAN"
    for op in dve_ops.OPS:
        if op.name == name:
            return op

    def ref(in0, in1, s0, s1, imm2):
        p0 = in0.astype(np.float32).reshape(in0.shape[0], -1)
        p1 = in1.astype(np.float32).reshape(in1.shape[0], -1)
        return np.cumsum(p0 * p1, axis=-1).reshape(in0.shape)

    spec = Spec(body=scan(AluOp.ADD, Src0 * Src1), reference=ref)
    op = dve_ops.DveOp(name, spec, subdim=False, uops_sha=dict(_WSUM_SHAS))
    dve_ops.OPS.append(op)
    dve_ops._SUB_OPCODE_FOR_NAME[name] = (
        max(dve_ops._SUB_OPCODE_FOR_NAME.values()) + 1)
    assert dve_ops._SUB_OPCODE_FOR_NAME[name] < 0x20
    return op


def _build():
    import concourse.bass as bass
    import concourse.mybir as mybir
    import concourse.tile as tile
    from concourse import bacc

    wsum = _register_wsum()

    nc = bacc.Bacc(None, target_bir_lowering=False, debug=False)
    # dateT packed [p, t, fc, j] = date[t*128+j, fc*128+p]: every t-range
    # DMA piece is contiguous in HBM with 1 KB runs, and the stationary
    # slice dall[:, t, fc, :] is a contiguous [128, 128] AP
    dh = nc.dram_tensor("dh", [P, NB, 4, P], mybir.dt.float16,
                        kind="ExternalInput")
    # fw slice 0 partition-major: fw0[p, fc*512+c] = fw2[fc*128+p, c]
    fw0 = nc.dram_tensor("fw0", [P, 4 * 512], mybir.dt.float16,
                         kind="ExternalInput")
    # fw slices 1..NF-1: fwn[(n-1)*128+p, fc*512+c] = fw2[fc*128+p, n*512+c]
    fwn = nc.dram_tensor("fwn", [(NF - 1) * P, 4 * 512], mybir.dt.float16,
                         kind="ExternalInput")
    # hybrid slices: f<256 half in fp16 pre-scaled by 2**14, f>=256 half
    # quantized to fp8e4 (scales 32*512 = 2**14); full-fp8 slices have all
    # four f-chunks in fp8.  The DVE weights for both carry the inverse
    # scale.
    fwh16 = nc.dram_tensor("fwh16", [NHYB * P, 2 * 512], mybir.dt.float16,
                           kind="ExternalInput")
    fwh8 = nc.dram_tensor("fwh8", [NHYB * P, 2 * 512], mybir.dt.float8e4,
                          kind="ExternalInput")
    fwf8 = nc.dram_tensor("fwf8", [NFULL * P, 4 * 512], mybir.dt.float8e4,
                          kind="ExternalInput")
    # date fp8 pairs: dh8[p, t, fc, m] = e4m3(date[t*128+m, fc*128+p]*32)
    dh8 = nc.dram_tensor("dh8", [P, NB, 4, P], mybir.dt.float8e4,
                         kind="ExternalInput")
    wdsh = nc.dram_tensor("wdsh", [P, NB * HSH], mybir.dt.float32,
                          kind="ExternalInput")
    # wdiff b-tiled: [p, t, h] = wdiff[t*128+p, h] -> contiguous col-splits
    wds = nc.dram_tensor("wds", [P, NB * HSH], mybir.dt.float32,
                         kind="ExternalInput")
    # prefix-sum segment ends; host differences them into per-c sums
    ends = nc.dram_tensor("ends", [B, C], mybir.dt.float32,
                          kind="ExternalOutput")

    with tile.TileContext(nc) as tc:
        with (
            tc.tile_pool(name="const", bufs=1) as cp,
            tc.tile_pool(name="fwp", bufs=6) as fwp,
            tc.tile_pool(name="wgp", bufs=8) as wgp,
            tc.tile_pool(name="ps", bufs=7, space="PSUM") as ps,
            tc.tile_pool(name="dps", bufs=1, space="PSUM") as dps,
        ):
            # resident fp16 dateT, wdiff rows, SBUF staging for segment ends
            dall = cp.tile([P, NB, 4, P], mybir.dt.float16, name="dall")
            f0 = cp.tile([P, 4 * 512], mybir.dt.float16, name="f0")
            # wdiff in two tiles so chain-0's DVE never depends on the
            # later wdiff DMA piece
            wra = cp.tile([P, 4, HSH], mybir.dt.float32, name="wra")
            wrc = cp.tile([P, NB - 4, HSH], mybir.dt.float32, name="wrc")
            es = [cp.tile([P, C], mybir.dt.float32, name=f"es{t}")
                  for t in range(NB)]
            dum = cp.tile([P, 256], mybir.dt.float16, name="dum")
            dum_ps = dps.tile([P, 256], mybir.dt.float32, name="dum_ps")
            # fp8-slice operands (resident; needed only from ~halfway)
            dall8 = cp.tile([P, NB, 4, P], mybir.dt.float8e4, name="dall8")
            fh16 = [cp.tile([P, 2 * 512], mybir.dt.float16, name=f"fh16_{j}")
                    for j in range(NHYB)]
            fh8 = [cp.tile([P, 2, 512], mybir.dt.float8e4, name=f"fh8_{j}")
                   for j in range(NHYB)]
            ff8 = [cp.tile([P, 4, 512], mybir.dt.float8e4, name=f"ff8_{j}")
                   for j in range(NFULL)]
            wrh = cp.tile([P, NB, HSH], mybir.dt.float32, name="wrh")

            # per-b-tile wdiff rows repeated 8x via a stride-0 middle dim
            wrb, wrbh = [], []
            for t in range(NB):
                s = (wra[:, t:t + 1, :] if t < 4
                     else wrc[:, t - 4:t - 3, :])
                wrb.append(bass.AP(s.tensor, s.offset,
                                   [s.ap[0], [0, 8], s.ap[-1]]))
                sh = wrh[:, t:t + 1, :]
                wrbh.append(bass.AP(sh.tensor, sh.offset,
                                    [sh.ap[0], [0, 8], sh.ap[-1]]))

            # HAM warmup: dummy matmuls on a zeroed tile span the startup
            # DMA wait so real matmuls begin at the full 2.4 GHz clock
            nc.gpsimd.memset(dum[:], 0.0)
            for _ in range(N_DUMMY):
                nc.tensor.matmul(dum_ps[:], dum[:, 0:128], dum[:],
                                 start=True, stop=True)

            # input loads interleaved across the two HWDGE queues (Sync +
            # Activation) so descriptor generation runs in parallel; pieces
            # split so each chain's dependencies land just in time
            nc.sync.dma_start(dall[:, 0, :, :], dh[:, 0, :, :])
            nc.scalar.dma_start(f0[:], fw0[:])
            nc.sync.dma_start(dall[:, 1, :, :], dh[:, 1, :, :])
            nc.scalar.dma_start(dall[:, 2:4, :, :], dh[:, 2:4, :, :])
            nc.sync.dma_start(wra[:], wds[:, 0:4 * HSH])
            nc.scalar.dma_start(dall[:, 4:8, :, :], dh[:, 4:8, :, :])
            nc.sync.dma_start(dall[:, 8:12, :, :], dh[:, 8:12, :, :])
            nc.scalar.dma_start(dall[:, 12:NB, :, :], dh[:, 12:NB, :, :])
            nc.sync.dma_start(wrc[:], wds[:, 4 * HSH:])

            # fw slice prefetch, one 512 KB trigger per slice, alternating
            # queues
            fts = [f0]
            for n in range(1, NF):
                ft = fwp.tile([P, 4 * 512], mybir.dt.float16, name="ft",
                              tag="ft")
                eng = nc.scalar if n % 2 else nc.sync
                eng.dma_start(ft[:], fwn[(n - 1) * P:n * P, :])
                fts.append(ft)
            # fp8-slice loads (const tiles, consumed from ~halfway on)
            nc.sync.dma_start(dall8[:], dh8[:])
            nc.scalar.dma_start(wrh[:], wdsh[:])
            for j in range(NHYB):
                eng = nc.scalar if j % 2 else nc.sync
                eng.dma_start(fh16[j][:], fwh16[j * P:(j + 1) * P, :])
                eng2 = nc.sync if j % 2 else nc.scalar
                eng2.dma_start(fh8[j][:], fwh8[j * P:(j + 1) * P, :])
            for j in range(NFULL):
                eng = nc.scalar if j % 2 else nc.sync
                eng.dma_start(ff8[j][:], fwf8[j * P:(j + 1) * P, :])

            def emit_chain(n, t):
                hyb = n in HYB_SLICES
                full = n in FULL_SLICES
                g = ps.tile([P, 512], mybir.dt.float32, name="g", tag="g")
                if hyb:
                    j = HYB_SLICES.index(n)
                    for fc in range(2):
                        nc.tensor.matmul(
                            g[:], dall[:, t, fc, :],
                            fh16[j][:, fc * 512:(fc + 1) * 512],
                            start=(fc == 0), stop=False)
                    nc.tensor.matmul(
                        g[:], dall8[:, t, 2:4, :], fh8[j][:],
                        start=False, stop=True,
                        perf_mode=mybir.MatmulPerfMode.DoubleRow)
                elif full:
                    j = FULL_SLICES.index(n)
                    for half in range(2):
                        nc.tensor.matmul(
                            g[:], dall8[:, t, 2 * half:2 * half + 2, :],
                            ff8[j][:, 2 * half:2 * half + 2, :],
                            start=(half == 0), stop=(half == 1),
                            perf_mode=mybir.MatmulPerfMode.DoubleRow)
                else:
                    j = F16_SLICES.index(n)
                    for fc in range(4):
                        nc.tensor.matmul(
                            g[:], dall[:, t, fc, :],
                            fts[j][:, fc * 512:(fc + 1) * 512],
                            start=(fc == 0), stop=(fc == 3))
                # weighted prefix sum over the tile in one DVE pass
                wg = wgp.tile([P, 8, HSH], mybir.dt.float32, name="wg",
                              tag="wg")
                nc.vector._custom_dve(wsum, out=wg[:], in0=g[:],
                                      in1=(wrbh if (hyb or full)
                                           else wrb)[t])
                # stage segment ends in SBUF (GPSIMD is otherwise idle)
                nc.gpsimd.tensor_copy(es[t][:, n * 8:(n + 1) * 8],
                                      wg[:, :, HSH - 1:HSH])
                if n == NS - 1:
                    nc.scalar.dma_start(ends[t * P:(t + 1) * P, :],
                                        es[t][:])

            # full-fp8 slices are chain-interleaved with the preceding fp16
            # slice: the fp16 chain's 864 ns of streaming hides the two
            # 256-column DoubleRow LDWEIGHTS, and the DVE sees a smoothed
            # (853+540)/2 cadence instead of a sustained 540 ns burst
            sched = [0, 1, 2, (3, 4), 5, 6, (7, 8), 9, 10, (11, 12),
                     13, 14, 15]
            for item in sched:
                if isinstance(item, tuple):
                    a, b = item
                    for t in range(NB):
                        emit_chain(a, t)
                        emit_chain(b, t)
                else:
                    for t in range(NB):
                        emit_chain(item, t)
    nc.finalize()
    return nc


def kernel(idx, date, train_dates, mem, train_nns, pos_w, pos_b, field_w,
           field_b, adapt_w, adapt_b):
    global _NC, _LAST_IN_MAPS
    from concourse.bass_utils import run_bass_kernel_spmd

    idx = np.asarray(idx)
    date = np.asarray(date, dtype=np.float32)
    train_dates = np.asarray(train_dates, dtype=np.float32)
    mem = np.asarray(mem, dtype=np.float32)
    train_nns = np.asarray(train_nns)
    pos_w = np.asarray(pos_w, dtype=np.float32)
    pos_b = np.asarray(pos_b, dtype=np.float32)
    field_w = np.asarray(field_w, dtype=np.float32)
    field_b = np.asarray(field_b, dtype=np.float32)
    adapt_w = np.asarray(adapt_w, dtype=np.float32)
    adapt_b = np.asarray(adapt_b, dtype=np.float32)

    # ---- host phase 1 (small): dist, wdiff, const terms ----
    refs = train_nns[idx]                                   # [B, K]
    pos_d = date @ pos_w + pos_b                            # [B, H]
    pos_r = (train_dates[refs.reshape(-1)] @ pos_w + pos_b).reshape(B, K, H)
    diff = pos_d[:, None, :] - pos_r                        # [B, K, H]
    norm = np.sqrt((diff * diff).sum(-1))                   # [B, K]
    m = norm.min(axis=1, keepdims=True)
    e = np.exp(m - norm)
    dist = e / e.sum(axis=1, keepdims=True)                 # [B, K]
    wdiff = np.einsum("bk,bkh->bh", dist, diff).astype(np.float32)
    wmem = np.einsum("bk,bkc->bc", dist, mem[refs]).astype(np.float32)
    const = wmem @ adapt_w + adapt_b + wdiff @ field_b.reshape(H, C)

    # ---- device phase 2: grad-term, TP over the 65536 dim ----
    if _NC is None:
        _NC = _build()
    import ml_dtypes
    f8dt = ml_dtypes.float8_e4m3

    def q8(x, scale):
        return np.clip(x * scale, -235.0, 235.0).astype(f8dt)

    dateT16 = date.T.astype(np.float16)                     # [F, B]
    date4 = dateT16.reshape(4, P, NB, P)                    # [fc, p, t, j]
    dh = np.ascontiguousarray(date4.transpose(1, 2, 0, 3))  # [p, t, fc, j]
    dateTf = date.T.reshape(4, P, NB, P)                    # fp32 [fc, p, t, j]
    dh8 = np.ascontiguousarray(
        q8(dateTf.transpose(1, 2, 0, 3), SC_D))             # [p, t, fc, m]
    fw3 = field_w.reshape(F, H, C)                          # [f, h, c]
    in_maps = []
    for i in range(NCORES):
        # c-major/h-minor columns for this core's h range
        fw2f = np.ascontiguousarray(
            fw3[:, i * HSH:(i + 1) * HSH, :].transpose(0, 2, 1)
        ).reshape(F, SH)                                    # col = c*64 + h
        fw4f = fw2f.reshape(4, P, NS, 512)                  # [fc, p, n, c]
        fw4 = fw4f.astype(np.float16)
        fw0 = np.ascontiguousarray(
            fw4[:, :, 0, :].transpose(1, 0, 2)).reshape(P, 4 * 512)
        fwn = np.ascontiguousarray(
            fw4[:, :, list(F16_SLICES[1:]), :].transpose(2, 1, 0, 3)
        ).reshape((NF - 1) * P, 4 * 512)
        fwh16 = np.ascontiguousarray(
            (fw4f[0:2, :, list(HYB_SLICES), :] * SC).transpose(2, 1, 0, 3)
        ).astype(np.float16).reshape(NHYB * P, 2 * 512)     # [n, p, fc, c]
        fwh8 = np.ascontiguousarray(
            q8(fw4f[2:4, :, list(HYB_SLICES), :], SC_W).transpose(2, 1, 0, 3)
        ).reshape(NHYB * P, 2 * 512)                        # [n, p, i, c]
        fwf8 = np.ascontiguousarray(
            q8(fw4f[:, :, list(FULL_SLICES), :], SC_W).transpose(2, 1, 0, 3)
        ).reshape(NFULL * P, 4 * 512)                       # [n, p, fc, c]
        # wdiff b-tiled [p, t, h] = wdiff[t*128+p, i*64+h]
        wdt = np.ascontiguousarray(
            wdiff[:, i * HSH:(i + 1) * HSH]
            .reshape(NB, P, HSH).transpose(1, 0, 2).reshape(P, NB * HSH))
        in_maps.append({
            "dh": dh,
            "dh8": dh8,
            "fw0": fw0,
            "fwn": fwn,
            "fwh16": fwh16,
            "fwh8": fwh8,
            "fwf8": fwf8,
            "wds": wdt,
            "wdsh": (wdt / np.float32(SC)).astype(np.float32),
        })
    _LAST_IN_MAPS = in_maps
    res = run_bass_kernel_spmd(_NC, in_maps, core_ids=list(range(NCORES)))
    grad_term = np.zeros((B, C), dtype=np.float32)
    for i in range(NCORES):
        e8 = res.results[i]["ends"].reshape(B, NS, 8)
        grad_term += np.diff(e8, axis=2, prepend=0.0).reshape(B, C)
    return (const + grad_term).astype(np.float32)


def run_device(trace=False):
    """Re-run the device phase on the last inputs (test.py profiling)."""
    from concourse.bass_utils import run_bass_kernel_spmd
    assert _NC is not None and _LAST_IN_MAPS is not None
    return run_bass_kernel_spmd(_NC, _LAST_IN_MAPS,
                                core_ids=list(range(NCORES)), trace=trace)


# revision 36
# speedup vs baseline: 1.0068x; 1.0054x over previous
"""TRN2 Bass kernel for nn_MAD_4612794876395 (retrieval_knn).

Math: with dist = softmax_k(-||pos_d - pos_r||) and sum_k dist = 1, the
reference output collapses to
    out[b,c] = wmem@adapt_w + adapt_b + wdiff@field_b.reshape(H,C)
             + sum_h wdiff[b,h] * (date@field_w)[b, h*C+c]
where wdiff[b,h] = sum_k dist[b,k]*diff[b,k,h].  The dominant term is the
137 GFLOP date@field_w product, computed on 8 NeuronCores tensor-parallel
over field_w's 65536 columns (64 h-values per core) as fp16 matmuls at
1 row/cycle.

The h-contraction (one multiply-add per matmul output element) is the
hard part: per-partition-scalar ops force 128-wide tiles and the three
elementwise engines together cannot sustain 4 such ops per 853ns matmul
chain.  Instead field_w columns are reordered c-major/h-minor on the
host so each PSUM tile is [128b, 8c x 64h], and a custom DVE op
(out = running sum of in0*in1, one elem/cycle) computes weighted prefix
sums in a single 512-wide pass; the 8 segment ends are DMA-extracted
and the host finishes with a cheap difference.  Small terms are host
numpy.

Optimizations over the first working version (242.5 us -> ~216 us):
 - Scheduling: the warm fp16 matmul stream runs at the 216 ns/MM
   hardware floor, so the wins are at the edges.  Startup loads are
   split into consumption-ordered pieces across both HWDGE queues
   (Sync + Activation) for parallel descriptor generation; every piece
   is a single large partition-major packed transfer (1-4 KB packets).
   A run of dummy matmuls on a zeroed tile spans the startup DMA wait
   so the HAM clock gate (PE at 1.2 GHz until ~3.4 us of sustained
   activity) is already open when the real matmuls start.
 - Mixed precision: the correctness budget (rel err < 2e-2) is spent on
   fp8 DoubleRow matmuls (2 fp8 MACs/cell/cycle).  3 of 16 column
   slices run all 512 f-channels in fp8e4 (two K=256-pair DR matmuls,
   ~540 ns/chain vs 853), 1 slice runs half fp16 / half fp8 (~663 ns).
   e4m3 operand scales 32 (date) x 512 (field_w) = 2**14 are folded
   into the pure-fp16 half (pre-scaled x2**14) and into a second
   wdiff/2**14 DVE weight set, so the PSUM accumulation stays
   consistent.  Measured rel err 1.77e-2 (numpy fp8 sim matches
   hardware to 4 digits); pure fp16 would be 2.9e-4.
 - The fp8 chains outpace the ~690 ns DVE prefix-sum, so fp8 slices are
   interleaved chain-by-chain with fp16 slices: the fp16 streaming
   hides the 256-column DR LDWEIGHTS and the DVE lag drains during
   fp16 groups instead of piling into the kernel tail.
"""
import sys

sys.path.insert(0, "/opt/trn_rl_repo")

import numpy as np

N_DATA, F, H, C, K, B = 100000, 512, 512, 128, 8, 2048
NCORES = 8
HSH = H // NCORES          # 64 h-values per core
SH = HSH * C               # 8192 field_w cols per core
P = 128
NB = B // P                # 16 b-tiles
NS = SH // 512             # 16 col-slices of 512 (8 c-segments x 64 h)
N_DUMMY = 22               # HAM warmup matmuls (N=256, ~213 ns each cold)
NHYB = 1                   # hybrid slices: f<256 fp16 + f>=256 fp8
NFULL = 3                  # full-fp8 slices: all 512 f in fp8 (2 DR MMs)
NH = NHYB + NFULL
NF = NS - NH               # pure-fp16 slices
# fp8 slices interleaved among fp16 ones: their chains outpace the DVE
# prefix-sum (~540/663 ns vs ~690 ns), so the DVE lag they build must
# drain during fp16 groups (853 ns cadence) before the next fp8 group
FULL_SLICES = (4, 8, 12)
HYB_SLICES = (14,)
F16_SLICES = tuple(n for n in range(NS)
                   if n not in FULL_SLICES and n not in HYB_SLICES)
SC_D, SC_W = 32.0, 512.0   # fp8 operand scales; product 2**14
SC = SC_D * SC_W

_NC = None
_LAST_IN_MAPS = None

_WSUM_SHAS = {"v3": "b3fc3e78a862b7eb", "v4": "bc6a002865d48b97"}


def _register_wsum():
    """Register the weighted-prefix-sum custom DVE op (idempotent)."""
    from concourse import dve_ops
    from concourse.dve_spec import Spec, Src0, Src1, scan, AluOp

    name = "ANT_WSUM_SCAN"
    for op in dve_ops.OPS:
        if op.name == name:
            return op

    def ref(in0, in1, s0, s1, imm2):
        p0 = in0.astype(np.float32).reshape(in0.shape[0], -1)
        p1 = in1.astype(np.float32).reshape(in1.shape[0], -1)
        return np.cumsum(p0 * p1, axis=-1).reshape(in0.shape)

    spec = Spec(body=scan(AluOp.ADD, Src0 * Src1), reference=ref)
    op = dve_ops.DveOp(name, spec, subdim=False, uops_sha=dict(_WSUM_SHAS))
    dve_ops.OPS.append(op)
    dve_ops._SUB_OPCODE_FOR_NAME[name] = (
        max(dve_ops._SUB_OPCODE_FOR_NAME.values()) + 1)
    assert dve_ops._SUB_OPCODE_FOR_NAME[name] < 0x20
    return op


def _build():
    import concourse.bass as bass
    import concourse.mybir as mybir
    import concourse.tile as tile
    from concourse import bacc

    wsum = _register_wsum()

    nc = bacc.Bacc(None, target_bir_lowering=False, debug=False)
    # dateT packed [p, t, fc, j] = date[t*128+j, fc*128+p]: every t-range
    # DMA piece is contiguous in HBM with 1 KB runs, and the stationary
    # slice dall[:, t, fc, :] is a contiguous [128, 128] AP
    dh = nc.dram_tensor("dh", [P, NB, 4, P], mybir.dt.float16,
                        kind="ExternalInput")
    # fw slice 0 partition-major: fw0[p, fc*512+c] = fw2[fc*128+p, c]
    fw0 = nc.dram_tensor("fw0", [P, 4 * 512], mybir.dt.float16,
                         kind="ExternalInput")
    # fw slices 1..NF-1: fwn[(n-1)*128+p, fc*512+c] = fw2[fc*128+p, n*512+c]
    fwn = nc.dram_tensor("fwn", [(NF - 1) * P, 4 * 512], mybir.dt.float16,
                         kind="ExternalInput")
    # hybrid slices: f<256 half in fp16 pre-scaled by 2**14, f>=256 half
    # quantized to fp8e4 (scales 32*512 = 2**14); full-fp8 slices have all
    # four f-chunks in fp8.  The DVE weights for both carry the inverse
    # scale.
    fwh16 = nc.dram_tensor("fwh16", [NHYB * P, 2 * 512], mybir.dt.float16,
                           kind="ExternalInput")
    fwh8 = nc.dram_tensor("fwh8", [NHYB * P, 2 * 512], mybir.dt.float8e4,
                          kind="ExternalInput")
    fwf8 = nc.dram_tensor("fwf8", [NFULL * P, 4 * 512], mybir.dt.float8e4,
                          kind="ExternalInput")
    # date fp8 pairs: dh8[p, t, fc, m] = e4m3(date[t*128+m, fc*128+p]*32)
    dh8 = nc.dram_tensor("dh8", [P, NB, 4, P], mybir.dt.float8e4,
                         kind="ExternalInput")
    wdsh = nc.dram_tensor("wdsh", [P, NB * HSH], mybir.dt.float32,
                          kind="ExternalInput")
    # wdiff b-tiled: [p, t, h] = wdiff[t*128+p, h] -> contiguous col-splits
    wds = nc.dram_tensor("wds", [P, NB * HSH], mybir.dt.float32,
                         kind="ExternalInput")
    # prefix-sum segment ends; host differences them into per-c sums
    ends = nc.dram_tensor("ends", [B, C], mybir.dt.float32,
                          kind="ExternalOutput")

    with tile.TileContext(nc) as tc:
        with (
            tc.tile_pool(name="const", bufs=1) as cp,
            tc.tile_pool(name="fwp", bufs=6) as fwp,
            tc.tile_pool(name="wgp", bufs=8) as wgp,
            tc.tile_pool(name="ps", bufs=7, space="PSUM") as ps,
            tc.tile_pool(name="dps", bufs=1, space="PSUM") as dps,
        ):
            # resident fp16 dateT, wdiff rows, SBUF staging for segment ends
            dall = cp.tile([P, NB, 4, P], mybir.dt.float16, name="dall")
            f0 = cp.tile([P, 4 * 512], mybir.dt.float16, name="f0")
            # wdiff in two tiles so chain-0's DVE never depends on the
            # later wdiff DMA piece
            wra = cp.tile([P, 4, HSH], mybir.dt.float32, name="wra")
            wrc = cp.tile([P, NB - 4, HSH], mybir.dt.float32, name="wrc")
            es = [cp.tile([P, C], mybir.dt.float32, name=f"es{t}")
                  for t in range(NB)]
            dum = cp.tile([P, 256], mybir.dt.float16, name="dum")
            dum_ps = dps.tile([P, 256], mybir.dt.float32, name="dum_ps")
            # fp8-slice operands (resident; needed only from ~halfway)
            dall8 = cp.tile([P, NB, 4, P], mybir.dt.float8e4, name="dall8")
            fh16 = [cp.tile([P, 2 * 512], mybir.dt.float16, name=f"fh16_{j}")
                    for j in range(NHYB)]
            fh8 = [cp.tile([P, 2, 512], mybir.dt.float8e4, name=f"fh8_{j}")
                   for j in range(NHYB)]
            ff8 = [cp.tile([P, 4, 512], mybir.dt.float8e4, name=f"ff8_{j}")
                   for j in range(NFULL)]
            wrh = cp.tile([P, NB, HSH], mybir.dt.float32, name="wrh")

            # per-b-tile wdiff rows repeated 8x via a stride-0 middle dim
            wrb, wrbh = [], []
            for t in range(NB):
                s = (wra[:, t:t + 1, :] if t < 4
                     else wrc[:, t - 4:t - 3, :])
                wrb.append(bass.AP(s.tensor, s.offset,
                                   [s.ap[0], [0, 8], s.ap[-1]]))
                sh = wrh[:, t:t + 1, :]
                wrbh.append(bass.AP(sh.tensor, sh.offset,
                                    [sh.ap[0], [0, 8], sh.ap[-1]]))

            # HAM warmup: dummy matmuls on a zeroed tile span the startup
            # DMA wait so real matmuls begin at the full 2.4 GHz clock
            nc.gpsimd.memset(dum[:], 0.0)
            for _ in range(N_DUMMY):
                nc.tensor.matmul(dum_ps[:], dum[:, 0:128], dum[:],
                                 start=True, stop=True)

            # input loads interleaved across the two HWDGE queues (Sync +
            # Activation) so descriptor generation runs in parallel; pieces
            # split so each chain's dependencies land just in time
            nc.sync.dma_start(dall[:, 0, :, :], dh[:, 0, :, :])
            nc.scalar.dma_start(f0[:], fw0[:])
            nc.sync.dma_start(dall[:, 1, :, :], dh[:, 1, :, :])
            nc.scalar.dma_start(dall[:, 2:4, :, :], dh[:, 2:4, :, :])
            nc.sync.dma_start(dall[:, 4:6, :, :], dh[:, 4:6, :, :])
            nc.scalar.dma_start(wra[:], wds[:, 0:4 * HSH])
            nc.sync.dma_start(dall[:, 6:9, :, :], dh[:, 6:9, :, :])
            nc.scalar.dma_start(dall[:, 9:12, :, :], dh[:, 9:12, :, :])
            nc.sync.dma_start(wrc[:], wds[:, 4 * HSH:])
            nc.scalar.dma_start(dall[:, 12:NB, :, :], dh[:, 12:NB, :, :])

            # fw slice prefetch, one 512 KB trigger per slice, alternating
            # queues
            fts = [f0]
            for n in range(1, NF):
                ft = fwp.tile([P, 4 * 512], mybir.dt.float16, name="ft",
                              tag="ft")
                eng = nc.scalar if n % 2 else nc.sync
                eng.dma_start(ft[:], fwn[(n - 1) * P:n * P, :])
                fts.append(ft)
            # fp8-slice loads (const tiles, consumed from ~halfway on)
            nc.sync.dma_start(dall8[:], dh8[:])
            nc.scalar.dma_start(wrh[:], wdsh[:])
            for j in range(NHYB):
                eng = nc.scalar if j % 2 else nc.sync
                eng.dma_start(fh16[j][:], fwh16[j * P:(j + 1) * P, :])
                eng2 = nc.sync if j % 2 else nc.scalar
                eng2.dma_start(fh8[j][:], fwh8[j * P:(j + 1) * P, :])
            for j in range(NFULL):
                eng = nc.scalar if j % 2 else nc.sync
                eng.dma_start(ff8[j][:], fwf8[j * P:(j + 1) * P, :])

            def emit_chain(n, t):
                hyb = n in HYB_SLICES
                full = n in FULL_SLICES
                g = ps.tile([P, 512], mybir.dt.float32, name="g", tag="g")
                if hyb:
                    j = HYB_SLICES.index(n)
                    for fc in range(2):
                        nc.tensor.matmul(
                            g[:], dall[:, t, fc, :],
                            fh16[j][:, fc * 512:(fc + 1) * 512],
                            start=(fc == 0), stop=False)
                    nc.tensor.matmul(
                        g[:], dall8[:, t, 2:4, :], fh8[j][:],
                        start=False, stop=True,
                        perf_mode=mybir.MatmulPerfMode.DoubleRow)
                elif full:
                    j = FULL_SLICES.index(n)
                    for half in range(2):
                        nc.tensor.matmul(
                            g[:], dall8[:, t, 2 * half:2 * half + 2, :],
                            ff8[j][:, 2 * half:2 * half + 2, :],
                            start=(half == 0), stop=(half == 1),
                            perf_mode=mybir.MatmulPerfMode.DoubleRow)
                else:
                    j = F16_SLICES.index(n)
                    for fc in range(4):
                        nc.tensor.matmul(
                            g[:], dall[:, t, fc, :],
                            fts[j][:, fc * 512:(fc + 1) * 512],
                            start=(fc == 0), stop=(fc == 3))
                # weighted prefix sum over the tile in one DVE pass
                wg = wgp.tile([P, 8, HSH], mybir.dt.float32, name="wg",
                              tag="wg")
                nc.vector._custom_dve(wsum, out=wg[:], in0=g[:],
                                      in1=(wrbh if (hyb or full)
                                           else wrb)[t])
                # stage segment ends in SBUF (GPSIMD is otherwise idle)
                nc.gpsimd.tensor_copy(es[t][:, n * 8:(n + 1) * 8],
                                      wg[:, :, HSH - 1:HSH])
                if n == NS - 1:
                    nc.scalar.dma_start(ends[t * P:(t + 1) * P, :],
                                        es[t][:])

            # full-fp8 slices are chain-interleaved with the preceding fp16
            # slice: the fp16 chain's 864 ns of streaming hides the two
            # 256-column DoubleRow LDWEIGHTS, and the DVE sees a smoothed
            # (853+540)/2 cadence instead of a sustained 540 ns burst
            sched = [0, 1, 2, (3, 4), 5, 6, (7, 8), 9, 10, (11, 12),
                     13, 14, 15]
            for item in sched:
                if isinstance(item, tuple):
                    a, b = item
                    for t in range(NB):
                        emit_chain(a, t)
                        emit_chain(b, t)
                else:
                    for t in range(NB):
                        emit_chain(item, t)
    nc.finalize()
    return nc


def kernel(idx, date, train_dates, mem, train_nns, pos_w, pos_b, field_w,
           field_b, adapt_w, adapt_b):
    global _NC, _LAST_IN_MAPS
    from concourse.bass_utils import run_bass_kernel_spmd

    idx = np.asarray(idx)
    date = np.asarray(date, dtype=np.float32)
    train_dates = np.asarray(train_dates, dtype=np.float32)
    mem = np.asarray(mem, dtype=np.float32)
    train_nns = np.asarray(train_nns)
    pos_w = np.asarray(pos_w, dtype=np.float32)
    pos_b = np.asarray(pos_b, dtype=np.float32)
    field_w = np.asarray(field_w, dtype=np.float32)
    field_b = np.asarray(field_b, dtype=np.float32)
    adapt_w = np.asarray(adapt_w, dtype=np.float32)
    adapt_b = np.asarray(adapt_b, dtype=np.float32)

    # ---- host phase 1 (small): dist, wdiff, const terms ----
    refs = train_nns[idx]                                   # [B, K]
    pos_d = date @ pos_w + pos_b                            # [B, H]
    pos_r = (train_dates[refs.reshape(-1)] @ pos_w + pos_b).reshape(B, K, H)
    diff = pos_d[:, None, :] - pos_r                        # [B, K, H]
    norm = np.sqrt((diff * diff).sum(-1))                   # [B, K]
    m = norm.min(axis=1, keepdims=True)
    e = np.exp(m - norm)
    dist = e / e.sum(axis=1, keepdims=True)                 # [B, K]
    wdiff = np.einsum("bk,bkh->bh", dist, diff).astype(np.float32)
    wmem = np.einsum("bk,bkc->bc", dist, mem[refs]).astype(np.float32)
    const = wmem @ adapt_w + adapt_b + wdiff @ field_b.reshape(H, C)

    # ---- device phase 2: grad-term, TP over the 65536 dim ----
    if _NC is None:
        _NC = _build()
    import ml_dtypes
    f8dt = ml_dtypes.float8_e4m3

    def q8(x, scale):
        return np.clip(x * scale, -235.0, 235.0).astype(f8dt)

    dateT16 = date.T.astype(np.float16)                     # [F, B]
    date4 = dateT16.reshape(4, P, NB, P)                    # [fc, p, t, j]
    dh = np.ascontiguousarray(date4.transpose(1, 2, 0, 3))  # [p, t, fc, j]
    dateTf = date.T.reshape(4, P, NB, P)                    # fp32 [fc, p, t, j]
    dh8 = np.ascontiguousarray(
        q8(dateTf.transpose(1, 2, 0, 3), SC_D))             # [p, t, fc, m]
    fw3 = field_w.reshape(F, H, C)                          # [f, h, c]
    in_maps = []
    for i in range(NCORES):
        # c-major/h-minor columns for this core's h range
        fw2f = np.ascontiguousarray(
            fw3[:, i * HSH:(i + 1) * HSH, :].transpose(0, 2, 1)
        ).reshape(F, SH)                                    # col = c*64 + h
        fw4f = fw2f.reshape(4, P, NS, 512)                  # [fc, p, n, c]
        fw4 = fw4f.astype(np.float16)
        fw0 = np.ascontiguousarray(
            fw4[:, :, 0, :].transpose(1, 0, 2)).reshape(P, 4 * 512)
        fwn = np.ascontiguousarray(
            fw4[:, :, list(F16_SLICES[1:]), :].transpose(2, 1, 0, 3)
        ).reshape((NF - 1) * P, 4 * 512)
        fwh16 = np.ascontiguousarray(
            (fw4f[0:2, :, list(HYB_SLICES), :] * SC).transpose(2, 1, 0, 3)
        ).astype(np.float16).reshape(NHYB * P, 2 * 512)     # [n, p, fc, c]
        fwh8 = np.ascontiguousarray(
            q8(fw4f[2:4, :, list(HYB_SLICES), :], SC_W).transpose(2, 1, 0, 3)
        ).reshape(NHYB * P, 2 * 512)                        # [n, p, i, c]
        fwf8 = np.ascontiguousarray(
            q8(fw4f[:, :, list(FULL_SLICES), :], SC_W).transpose(2, 1, 0, 3)
        ).reshape(NFULL * P, 4 * 512)                       # [n, p, fc, c]
        # wdiff b-tiled [p, t, h] = wdiff[t*128+p, i*64+h]
        wdt = np.ascontiguousarray(
            wdiff[:, i * HSH:(i + 1) * HSH]
            .reshape(NB, P, HSH).transpose(1, 0, 2).reshape(P, NB * HSH))
        in_maps.append({
            "dh": dh,
            "dh8": dh8,
            "fw0": fw0,
            "fwn": fwn,
            "fwh16": fwh16,
            "fwh8": fwh8,
            "fwf8": fwf8,
            "wds": wdt,
            "wdsh": (wdt / np.float32(SC)).astype(np.float32),
        })
    _LAST_IN_MAPS = in_maps
    res = run_bass_kernel_spmd(_NC, in_maps, core_ids=list(range(NCORES)))
    grad_term = np.zeros((B, C), dtype=np.float32)
    for i in range(NCORES):
        e8 = res.results[i]["ends"].reshape(B, NS, 8)
        grad_term += np.diff(e8, axis=2, prepend=0.0).reshape(B, C)
    return (const + grad_term).astype(np.float32)


def run_device(trace=False):
    """Re-run the device phase on the last inputs (test.py profiling)."""
    from concourse.bass_utils import run_bass_kernel_spmd
    assert _NC is not None and _LAST_IN_MAPS is not None
    return run_bass_kernel_spmd(_NC, _LAST_IN_MAPS,
                                core_ids=list(range(NCORES)), trace=trace)
